# revision 3
# baseline (speedup 1.0000x reference)
"""Trainium2 Bass kernel for FASTMultiHeadAttention (degree-2 Taylor softmax
approximation with relative position bias).

  s_ij  = q_i . k_j + q_i . rpe[i-j+N-1]
  score = 1 + s + 0.5 s^2
  o_i   = sum_j score_ij v_j / sum_j score_ij

Sharding: batch*head (16 heads) split over 8 cores, 2 heads per core.

Per-core dataflow (per head h, per 128-row i-block):
  - PE: P'[p,t] = q_i . rpe_f[c0+t]  (windowed 1152-col matmul vs flipped rpe)
  - copy P' PSUM->SBUF (bf16), DMA to DRAM scratch
  - DMA sheared read back: G[p,j] = P'[p, 127+j-p]  (row stride 1151) --
    this realizes the relative-position diagonal gather as a plain strided DMA
  - PE: qk = Q_blk @ K^T;  DVE: t = qk + G (bf16)
  - PE transpose t in 128x128 chunks; ACT: st = Square(sqrt(.5)t + sqrt(.5))
    = 0.5(t+1)^2  (so score = st + 0.5, handled via correction matmul)
  - PE: O_psum[i, 0:65] = sum_chunks st^T @ [V | 1]  + ones_row x 0.5*vsum
  - DVE reciprocal of col 64; ACT scales cols 0:64; DMA out.
"""

import numpy as np
import ml_dtypes
from contextlib import ExitStack

import concourse.bass as bass
import concourse.mybir as mybir
import concourse.tile as tile
from concourse import bacc, bass_utils
from concourse.masks import make_identity

B, H, N, D = 2, 8, 1024, 64
BH = B * H
NCORES = 8
HPC = BH // NCORES  # heads per core
NB = N // 128       # i-blocks per head
W = 1152            # P' window width (1151 needed, padded to chunk multiple)
BF = mybir.dt.bfloat16
F32 = mybir.dt.float32
SQH = float(np.sqrt(0.5))
BF_NP = ml_dtypes.bfloat16

TRACE = False
_cached_nc = None


def _build():
    nc = bacc.Bacc("TRN2", target_bir_lowering=False, debug=False,
                   num_devices=NCORES)
    qt = nc.dram_tensor("qt", [HPC, D, N], BF, kind="ExternalInput").ap()
    kt = nc.dram_tensor("kt", [HPC, D, N], BF, kind="ExternalInput").ap()
    v = nc.dram_tensor("v", [HPC, N, D], BF, kind="ExternalInput").ap()
    rpet = nc.dram_tensor("rpet", [D, 2048], BF, kind="ExternalInput").ap()
    o = nc.dram_tensor("o", [HPC, N, D], F32, kind="ExternalOutput").ap()
    scr = nc.dram_tensor("scr", [HPC * NB * 128 * W], BF, kind="Internal")

    with tile.TileContext(nc) as tc, ExitStack() as ctx:
        const = ctx.enter_context(tc.tile_pool(name="const", bufs=1))
        vpool = ctx.enter_context(tc.tile_pool(name="vpool", bufs=2))
        work = ctx.enter_context(tc.tile_pool(name="work", bufs=3))
        stp = ctx.enter_context(tc.tile_pool(name="stp", bufs=4))
        outp = ctx.enter_context(tc.tile_pool(name="outp", bufs=4))
        pp = ctx.enter_context(tc.tile_pool(name="pp", bufs=2, space="PSUM"))
        pqk = ctx.enter_context(tc.tile_pool(name="pqk", bufs=2, space="PSUM"))
        pt = ctx.enter_context(tc.tile_pool(name="pt", bufs=2, space="PSUM"))
        po = ctx.enter_context(tc.tile_pool(name="po", bufs=1, space="PSUM"))

        ident = const.tile([128, 128], BF, tag="ident")
        make_identity(nc, ident[:])
        ones_col = const.tile([128, 1], BF, tag="onec")
        nc.vector.memset(ones_col[:], 1.0)
        ones_row = const.tile([1, 128], BF, tag="oner")
        nc.vector.memset(ones_row[:], 1.0)
        sqh_bias = const.tile([128, 1], F32, tag="sqhb")
        nc.vector.memset(sqh_bias[:], SQH)

        qt_sb = const.tile([D, HPC, N], BF, tag="qt")
        nc.sync.dma_start(qt_sb[:], qt.rearrange("h d n -> d h n"))
        kt_sb = const.tile([D, HPC, N], BF, tag="kt")
        nc.sync.dma_start(kt_sb[:], kt.rearrange("h d n -> d h n"))
        rp_sb = const.tile([D, 2048], BF, tag="rp")
        nc.sync.dma_start(rp_sb[:], rpet)

        for h in range(HPC):
            # V chunks with appended ones column: [128, chunk, 65]
            vaug = vpool.tile([128, NB, 65], BF, tag="vaug")
            nc.sync.dma_start(
                vaug[:, :, 0:64], v[h].rearrange("(c p) d -> p c d", p=128))
            nc.vector.memset(vaug[:, :, 64], 1.0)

            # vsum[0, :] = [colsum(V) | N]; store 0.5x in SBUF
            vs_psum = pqk.tile([128, 512], F32, tag="pqk")
            for c in range(NB):
                nc.tensor.matmul(vs_psum[0:1, 0:65], ones_col[:],
                                 vaug[:, c, :], start=(c == 0),
                                 stop=(c == NB - 1))
            vsum_sb = vpool.tile([1, 65], BF, tag="vsum")
            nc.scalar.activation(vsum_sb[:], vs_psum[0:1, 0:65],
                                 mybir.ActivationFunctionType.Copy, scale=0.5)

            for bi in range(NB):
                i0 = 128 * bi
                c0 = 896 - i0
                qblk = qt_sb[:, h, i0:i0 + 128]

                # ---- P' = Q_blk @ rpe_f^T over window [c0, c0+W) ----
                p_sb = work.tile([128, W], BF, tag="p")
                for ci, (off, wid) in enumerate(((0, 512), (512, 512),
                                                 (1024, 128))):
                    pps = pp.tile([128, 512], F32, tag="pp")
                    nc.tensor.matmul(pps[:, :wid], qblk,
                                     rp_sb[:, c0 + off:c0 + off + wid],
                                     start=True, stop=True)
                    # split PSUM->SBUF copies between ACT and DVE
                    if ci == 1:
                        nc.vector.tensor_copy(p_sb[:, off:off + wid],
                                              pps[:, :wid])
                    else:
                        nc.scalar.activation(
                            p_sb[:, off:off + wid], pps[:, :wid],
                            mybir.ActivationFunctionType.Copy)

                # ---- scratch round trip with sheared read ----
                base = (h * NB + bi) * 128 * W
                scr_w = bass.AP(scr, base, [[W, 128], [1, W]])
                nc.sync.dma_start(scr_w, p_sb[:])
                g_sb = work.tile([128, N], BF, tag="g")
                scr_r = bass.AP(scr, base + 127, [[W - 1, 128], [1, N]])
                nc.sync.dma_start(g_sb[:], scr_r)

                # ---- qk & t = qk + G ----
                t_sb = work.tile([128, N], BF, tag="t")
                for jc in range(2):
                    qkp = pqk.tile([128, 512], F32, tag="pqk")
                    nc.tensor.matmul(qkp[:], qblk,
                                     kt_sb[:, h, 512 * jc:512 * (jc + 1)],
                                     start=True, stop=True)
                    nc.vector.tensor_add(t_sb[:, 512 * jc:512 * (jc + 1)],
                                         qkp[:], g_sb[:, 512 * jc:512 * (jc + 1)])

                # ---- transpose, square, PV accumulate ----
                opsum = po.tile([128, 65], F32, tag="po")
                for c in range(NB):
                    tt = pt.tile([128, 128], BF, tag="pt")
                    nc.tensor.transpose(tt[:], t_sb[:, 128 * c:128 * (c + 1)],
                                        ident[:])
                    st = stp.tile([128, 128], BF, tag="st")
                    nc.scalar.activation(st[:], tt[:],
                                         mybir.ActivationFunctionType.Square,
                                         bias=sqh_bias[:], scale=SQH)
                    nc.tensor.matmul(opsum[:], st[:], vaug[:, c, :],
                                     start=(c == 0), stop=False)
                # correction row: += 1s^T x (0.5*[colsumV | N])
                nc.tensor.matmul(opsum[:], ones_row[:], vsum_sb[:],
                                 start=False, stop=True)

                # ---- normalize & store ----
                recip = outp.tile([128, 1], F32, tag="recip")
                nc.vector.reciprocal(recip[:], opsum[:, 64:65])
                o_sb = outp.tile([128, 64], F32, tag="osb")
                nc.scalar.activation(o_sb[:], opsum[:, 0:64],
                                     mybir.ActivationFunctionType.Copy,
                                     scale=recip[:])
                nc.sync.dma_start(o[h, i0:i0 + 128, :], o_sb[:])

    nc.compile()
    return nc


def kernel(**inputs):
    global _cached_nc
    q = np.asarray(inputs["q"], dtype=np.float32)
    k = np.asarray(inputs["k"], dtype=np.float32)
    v = np.asarray(inputs["v"], dtype=np.float32)
    rpe = np.asarray(inputs["rpe_matrix"], dtype=np.float32)

    qf = q.reshape(BH, N, D)
    kf = k.reshape(BH, N, D)
    vf = v.reshape(BH, N, D).astype(BF_NP)
    qt = np.ascontiguousarray(qf.transpose(0, 2, 1)).astype(BF_NP)
    kt = np.ascontiguousarray(kf.transpose(0, 2, 1)).astype(BF_NP)
    rpet = np.zeros((D, 2048), dtype=BF_NP)
    rpet[:, :2047] = np.ascontiguousarray(rpe[::-1].T).astype(BF_NP)

    if _cached_nc is None:
        _cached_nc = _build()
    nc = _cached_nc

    in_maps = []
    for c in range(NCORES):
        hs = slice(c * HPC, (c + 1) * HPC)
        in_maps.append({"qt": qt[hs], "kt": kt[hs], "v": vf[hs],
                        "rpet": rpet})

    res = bass_utils.run_bass_kernel_spmd(
        nc, in_maps, core_ids=list(range(NCORES)), trace=TRACE)
    if TRACE:
        print(f"HW exec time: {res.exec_time_ns} ns")
        if res.instructions_and_trace is not None:
            print("trace:", res.instructions_and_trace[1])

    o = np.concatenate([r["o"] for r in res.results], axis=0)
    return o.reshape(B, H, N, D).astype(np.float32)


# revision 7
# speedup vs baseline: 1138.1176x; 1138.1176x over previous
"""Trainium2 Bass kernel for FASTMultiHeadAttention (degree-2 Taylor softmax
approximation with relative position bias).

  s_ij  = q_i . k_j + q_i . rpe[i-j+N-1]
  score = 1 + s + 0.5 s^2
  o_i   = sum_j score_ij v_j / sum_j score_ij

Sharding: batch*head (16 heads) split over 8 cores, 2 heads per core.

Per-core dataflow (per head h, per 128-row i-block):
  - PE: P'[p,t] = q_i . rpe_f[c0+t]  (windowed 1152-col matmul vs flipped rpe)
  - copy P' PSUM->SBUF (bf16), DMA to DRAM scratch
  - DMA sheared read back: G[p,j] = P'[p, 127+j-p]  (row stride 1151) --
    this realizes the relative-position diagonal gather as a plain strided DMA
  - PE: qk = Q_blk @ K^T;  DVE: t = qk + G (bf16)
  - PE transpose t in 128x128 chunks; ACT: st = Square(sqrt(.5)t + sqrt(.5))
    = 0.5(t+1)^2  (so score = st + 0.5, handled via correction matmul)
  - PE: O_psum[i, 0:65] = sum_chunks st^T @ [V | 1]  + ones_row x 0.5*vsum
  - DVE reciprocal of col 64; ACT scales cols 0:64; DMA out.
"""

import numpy as np
import ml_dtypes
from contextlib import ExitStack

import concourse.bass as bass
import concourse.mybir as mybir
import concourse.tile as tile
from concourse import bacc, bass_utils
from concourse.masks import make_identity

B, H, N, D = 2, 8, 1024, 64
BH = B * H
NCORES = 8
HPC = BH // NCORES  # heads per core
NB = N // 128       # i-blocks per head
W = 1152            # P' window width (1151 needed, padded to chunk multiple)
BF = mybir.dt.bfloat16
F32 = mybir.dt.float32
SQH = float(np.sqrt(0.5))
BF_NP = ml_dtypes.bfloat16

TRACE = False
_cached_nc = None


def _build(repeat=1):
    nc = bacc.Bacc("TRN2", target_bir_lowering=False, debug=False,
                   num_devices=NCORES)
    qt = nc.dram_tensor("qt", [HPC, D, N], BF, kind="ExternalInput").ap()
    kt = nc.dram_tensor("kt", [HPC, D, N], BF, kind="ExternalInput").ap()
    v = nc.dram_tensor("v", [HPC, N, D], BF, kind="ExternalInput").ap()
    rpet = nc.dram_tensor("rpet", [D, 2048], BF, kind="ExternalInput").ap()
    o = nc.dram_tensor("o", [HPC, N, D], F32, kind="ExternalOutput").ap()
    scr = nc.dram_tensor("scr", [HPC * NB * 128 * W], BF, kind="Internal")

    with tile.TileContext(nc) as tc, ExitStack() as ctx:
        const = ctx.enter_context(tc.tile_pool(name="const", bufs=1))
        vpool = ctx.enter_context(tc.tile_pool(name="vpool", bufs=2))
        work = ctx.enter_context(tc.tile_pool(name="work", bufs=3))
        stp = ctx.enter_context(tc.tile_pool(name="stp", bufs=4))
        outp = ctx.enter_context(tc.tile_pool(name="outp", bufs=4))
        pp = ctx.enter_context(tc.tile_pool(name="pp", bufs=2, space="PSUM"))
        pqk = ctx.enter_context(tc.tile_pool(name="pqk", bufs=2, space="PSUM"))
        pt = ctx.enter_context(tc.tile_pool(name="pt", bufs=2, space="PSUM"))
        po = ctx.enter_context(tc.tile_pool(name="po", bufs=1, space="PSUM"))

        ident = const.tile([128, 128], BF, tag="ident")
        make_identity(nc, ident[:])
        ones_col = const.tile([128, 1], BF, tag="onec")
        nc.vector.memset(ones_col[:], 1.0)
        ones_row = const.tile([1, 128], BF, tag="oner")
        nc.vector.memset(ones_row[:], 1.0)
        sqh_bias = const.tile([128, 1], F32, tag="sqhb")
        nc.vector.memset(sqh_bias[:], SQH)

        qt_sb = const.tile([D, HPC, N], BF, tag="qt")
        nc.sync.dma_start(qt_sb[:], qt.rearrange("h d n -> d h n"))
        kt_sb = const.tile([D, HPC, N], BF, tag="kt")
        nc.sync.dma_start(kt_sb[:], kt.rearrange("h d n -> d h n"))
        rp_sb = const.tile([D, 2048], BF, tag="rp")
        nc.sync.dma_start(rp_sb[:], rpet)

        rep_ctx = tc.For_i(0, repeat, 1) if repeat > 1 else None
        if rep_ctx is not None:
            rep_ctx.__enter__()
        for _rep in range(1):
          for h in range(HPC):
            # V chunks with appended ones column: [128, chunk, 65]
            vaug = vpool.tile([128, NB, 65], BF, tag="vaug")
            nc.sync.dma_start(
                vaug[:, :, 0:64], v[h].rearrange("(c p) d -> p c d", p=128))
            nc.vector.memset(vaug[:, :, 64], 1.0)

            # vsum[0, :] = [colsum(V) | N]; store 0.5x in SBUF
            vs_psum = pqk.tile([128, 512], F32, tag="pqk")
            for c in range(NB):
                nc.tensor.matmul(vs_psum[0:1, 0:65], ones_col[:],
                                 vaug[:, c, :], start=(c == 0),
                                 stop=(c == NB - 1))
            vsum_sb = vpool.tile([1, 65], BF, tag="vsum")
            nc.scalar.activation(vsum_sb[:], vs_psum[0:1, 0:65],
                                 mybir.ActivationFunctionType.Copy, scale=0.5)

            for bi in range(NB):
                i0 = 128 * bi
                c0 = 896 - i0
                qblk = qt_sb[:, h, i0:i0 + 128]

                # ---- P' = Q_blk @ rpe_f^T over window [c0, c0+W) ----
                p_sb = work.tile([128, W], BF, tag="p")
                for ci, (off, wid) in enumerate(((0, 512), (512, 512),
                                                 (1024, 128))):
                    pps = pp.tile([128, 512], F32, tag="pp")
                    nc.tensor.matmul(pps[:, :wid], qblk,
                                     rp_sb[:, c0 + off:c0 + off + wid],
                                     start=True, stop=True)
                    # split PSUM->SBUF copies between ACT and DVE
                    if ci == 1:
                        nc.vector.tensor_copy(p_sb[:, off:off + wid],
                                              pps[:, :wid])
                    else:
                        nc.scalar.activation(
                            p_sb[:, off:off + wid], pps[:, :wid],
                            mybir.ActivationFunctionType.Copy)

                # ---- scratch round trip with sheared read ----
                base = (h * NB + bi) * 128 * W
                scr_w = bass.AP(scr, base, [[W, 128], [1, W]])
                nc.sync.dma_start(scr_w, p_sb[:])
                g_sb = work.tile([128, N], BF, tag="g")
                scr_r = bass.AP(scr, base + 127, [[W - 1, 128], [1, N]])
                nc.sync.dma_start(g_sb[:], scr_r)

                # ---- qk & t = qk + G ----
                t_sb = work.tile([128, N], BF, tag="t")
                for jc in range(2):
                    qkp = pqk.tile([128, 512], F32, tag="pqk")
                    nc.tensor.matmul(qkp[:], qblk,
                                     kt_sb[:, h, 512 * jc:512 * (jc + 1)],
                                     start=True, stop=True)
                    nc.vector.tensor_add(t_sb[:, 512 * jc:512 * (jc + 1)],
                                         qkp[:], g_sb[:, 512 * jc:512 * (jc + 1)])

                # ---- transpose, square, PV accumulate ----
                opsum = po.tile([128, 65], F32, tag="po")
                for c in range(NB):
                    tt = pt.tile([128, 128], BF, tag="pt")
                    nc.tensor.transpose(tt[:], t_sb[:, 128 * c:128 * (c + 1)],
                                        ident[:])
                    st = stp.tile([128, 128], BF, tag="st")
                    nc.scalar.activation(st[:], tt[:],
                                         mybir.ActivationFunctionType.Square,
                                         bias=sqh_bias[:], scale=SQH)
                    nc.tensor.matmul(opsum[:], st[:], vaug[:, c, :],
                                     start=(c == 0), stop=False)
                # correction row: += 1s^T x (0.5*[colsumV | N])
                nc.tensor.matmul(opsum[:], ones_row[:], vsum_sb[:],
                                 start=False, stop=True)

                # ---- normalize & store ----
                recip = outp.tile([128, 1], F32, tag="recip")
                nc.vector.reciprocal(recip[:], opsum[:, 64:65])
                o_sb = outp.tile([128, 64], F32, tag="osb")
                nc.scalar.activation(o_sb[:], opsum[:, 0:64],
                                     mybir.ActivationFunctionType.Copy,
                                     scale=recip[:])
                nc.sync.dma_start(o[h, i0:i0 + 128, :], o_sb[:])

        if rep_ctx is not None:
            rep_ctx.__exit__(None, None, None)

    nc.compile()
    return nc


def kernel(**inputs):
    global _cached_nc
    q = np.asarray(inputs["q"], dtype=np.float32)
    k = np.asarray(inputs["k"], dtype=np.float32)
    v = np.asarray(inputs["v"], dtype=np.float32)
    rpe = np.asarray(inputs["rpe_matrix"], dtype=np.float32)

    qf = q.reshape(BH, N, D)
    kf = k.reshape(BH, N, D)
    vf = v.reshape(BH, N, D).astype(BF_NP)
    qt = np.ascontiguousarray(qf.transpose(0, 2, 1)).astype(BF_NP)
    kt = np.ascontiguousarray(kf.transpose(0, 2, 1)).astype(BF_NP)
    rpet = np.zeros((D, 2048), dtype=BF_NP)
    rpet[:, :2047] = np.ascontiguousarray(rpe[::-1].T).astype(BF_NP)

    if _cached_nc is None:
        _cached_nc = _build()
    nc = _cached_nc

    in_maps = []
    for c in range(NCORES):
        hs = slice(c * HPC, (c + 1) * HPC)
        in_maps.append({"qt": qt[hs], "kt": kt[hs], "v": vf[hs],
                        "rpet": rpet})

    res = bass_utils.run_bass_kernel_spmd(
        nc, in_maps, core_ids=list(range(NCORES)), trace=TRACE)
    if TRACE:
        print(f"HW exec time: {res.exec_time_ns} ns")
        if res.instructions_and_trace is not None:
            print("trace:", res.instructions_and_trace[1])

    o = np.concatenate([r["o"] for r in res.results], axis=0)
    return o.reshape(B, H, N, D).astype(np.float32)
